# revision 10
# baseline (speedup 1.0000x reference)
"""MinGRU cell kernel for Trainium2 (8 NeuronCores, data-parallel over batch).

Reference computation (per sample n):
    zh = x[n] @ W.T + b            # (L, 2H)
    z, u = split(zh)               # each (L, H)
    s = sigmoid(z); a = 1 - s
    g = relu(u) + min(sigmoid(u), 0.5)      # == x+0.5 for x>=0, sigmoid(x) else
    h_t = a_t * h_{t-1} + s_t * g_t         # first-order linear recurrence
The reference evaluates the recurrence in log space (Heinsen scan) for
stability; in linear space it is a convex combination of positive values, so
direct evaluation is stable and matches to ~3e-4 scale-relative (the log-space
f32 cumsum noise of the reference itself dominates the difference).

Device mapping (per core = one batch sample):
  - matmul on PE in float32r (full-rate fp32): out[h, l] tiles, H on
    partitions, L on free dim -- exactly the layout tensor_tensor_scan needs
  - sigmoid/relu epilogues on ACT (reads PSUM, per-partition bias)
  - a and g elementwise on DVE, bval on the otherwise-idle gpsimd
  - recurrence via the hardware scan op (state = a*state + b along free dim)
  - x / W are pre-transposed on the host so no on-device transposes at all;
    the output is produced as h.T per sample and transposed back on the host.
"""

import sys
import numpy as np
import ml_dtypes

if "/opt/trn_rl_repo" not in sys.path:
    sys.path.insert(0, "/opt/trn_rl_repo")

from contextlib import ExitStack

import concourse.bass as bass
import concourse.mybir as mybir
import concourse.tile as tile
from concourse import bass_utils
from concourse.bass_utils import run_bass_kernel_spmd

P = 128
N_CORES = 8
L = 4096
H = 1024
HIN = 1024
KC = HIN // P      # contraction chunks (8)
HC = H // P        # hidden chunks per half (8)
LT = 512           # L tile (free dim per matmul / scan)
NLT = L // LT

F32 = mybir.dt.float32
F32R = mybir.dt.float32r
BF16 = mybir.dt.bfloat16
AF = mybir.ActivationFunctionType
OP = mybir.AluOpType


def split_waits(nc, max_waits=1):
    """This walrus build only supports one sync wait per instruction; move
    extras onto preceding no-ops on the same engine."""
    for func in nc.m.functions:
        for b in func.blocks:
            idx = 0
            while idx < len(b.instructions):
                inst = b.instructions[idx]
                si = inst.sync_info
                if si is not None and len(si.on_wait) > max_waits:
                    waits = list(si.on_wait)
                    pre, keep = waits[:-max_waits], waits[-max_waits:]
                    pos = idx
                    while pre:
                        chunk, pre = pre[:max_waits], pre[max_waits:]
                        nop = mybir.InstNoOp(
                            name=nc.get_next_instruction_name(), ins=[], outs=[])
                        nop.engine = inst.engine
                        nop.sync_info = mybir.SyncInfo(on_wait=chunk, on_update=[])
                        nc.register_instruction(nop)
                        b.instructions.insert(pos, nop)
                        pos += 1
                        idx += 1
                    si.on_wait = keep
                idx += 1


def build_program():
    nc = bass.Bass()
    xt = nc.dram_tensor("xt", [HIN, L], BF16, kind="ExternalInput")
    wt = nc.dram_tensor("wt", [HIN, 2 * H], BF16, kind="ExternalInput")
    bz = nc.dram_tensor("bz", [P, HC], F32, kind="ExternalInput")
    bh = nc.dram_tensor("bh", [P, HC], F32, kind="ExternalInput")
    h0 = nc.dram_tensor("h0", [P, HC], F32, kind="ExternalInput")
    ht = nc.dram_tensor("ht", [H, L], F32, kind="ExternalOutput")

    with tile.TileContext(nc) as tc:
        with ExitStack() as ctx:
            pool = lambda name, bufs: ctx.enter_context(
                tc.tile_pool(name=name, bufs=bufs))
            w_pool = pool("w", 1)
            bias_pool = pool("bias", 1)
            xt_pool = pool("xt", 3)
            s_pool = pool("s", 3)
            a_pool = pool("a", 3)
            sg_pool = pool("sg", 3)
            u_pool = pool("u", 3)
            bv_pool = pool("bv", 3)
            h_pool = pool("h", 2)
            psum = ctx.enter_context(
                tc.tile_pool(name="psum", bufs=4, space="PSUM"))

            # biases on the ACT hwdge queue (tiny); x rides the SP queue so
            # the two DMA queues fill in parallel at startup
            bz_sb = bias_pool.tile([P, HC], F32)
            nc.scalar.dma_start(bz_sb[:], bz[:])
            bh_sb = bias_pool.tile([P, HC], F32)
            nc.scalar.dma_start(bh_sb[:], bh[:])
            h0_sb = bias_pool.tile([P, HC], F32)
            nc.scalar.dma_start(h0_sb[:], h0[:])

            def load_x(l0, lw):
                tiles = []
                for ko in range(KC):
                    x_k = xt_pool.tile([P, lw], BF16, tag=f"x{ko}")
                    nc.sync.dma_start(
                        x_k[:], xt[ko * P:(ko + 1) * P, l0:l0 + lw])
                    tiles.append(x_k)
                return tiles

            # schedule: full 512-wide L-tiles, the last one split in half so
            # the epilogue drain after the final matmul is ~2x shorter
            sched = [(i * LT, LT) for i in range(NLT - 1)]
            sched.append(((NLT - 1) * LT, LT // 2))
            sched.append(((NLT - 1) * LT + LT // 2, LT // 2))

            # first x L-tile on the SP queue; W streams on the ACT queue with
            # the first two c-chunks of each half leading so the first matmul
            # groups start after ~1 MiB per queue
            x_first = load_x(0, LT)
            w_z, w_u = [], []
            for ko in range(KC):
                wz_k = w_pool.tile([P, H], BF16, tag=f"wz{ko}")
                nc.scalar.dma_start(
                    wz_k[:, 0:2 * P], wt[ko * P:(ko + 1) * P, 0:2 * P])
                w_z.append(wz_k)
            for ko in range(KC):
                wu_k = w_pool.tile([P, H], BF16, tag=f"wu{ko}")
                nc.scalar.dma_start(
                    wu_k[:, 0:2 * P], wt[ko * P:(ko + 1) * P, H:H + 2 * P])
                w_u.append(wu_k)
            for ko in range(KC):
                nc.scalar.dma_start(
                    w_z[ko][:, 2 * P:H], wt[ko * P:(ko + 1) * P, 2 * P:H])
            for ko in range(KC):
                nc.scalar.dma_start(
                    w_u[ko][:, 2 * P:H],
                    wt[ko * P:(ko + 1) * P, H + 2 * P:2 * H])

            def epilogue(c, l0, lw, z_ps, u_ps, first):
                s_sb = s_pool.tile([P, lw], F32, tag="s")
                nc.scalar.activation(
                    s_sb[:], z_ps[:], AF.Sigmoid, bias=bz_sb[:, c:c + 1])
                sg_sb = sg_pool.tile([P, lw], F32, tag="sg")
                nc.scalar.activation(
                    sg_sb[:], u_ps[:], AF.Sigmoid, bias=bh_sb[:, c:c + 1])
                # r = relu(u + bias)
                u_sb = u_pool.tile([P, lw], F32, tag="u")
                nc.scalar.activation(
                    u_sb[:], u_ps[:], AF.Relu, bias=bh_sb[:, c:c + 1])

                # a = 1 - s
                a_sb = a_pool.tile([P, lw], F32, tag="a")
                nc.vector.tensor_scalar(
                    a_sb[:], s_sb[:], -1.0, 1.0, OP.mult, OP.add)
                # g = min(sigmoid(u), 0.5) + relu(u)
                nc.vector.scalar_tensor_tensor(
                    u_sb[:], sg_sb[:], 0.5, u_sb[:], OP.min, OP.add)
                # bval = s * g   (on the otherwise-idle gpsimd engine)
                bv_sb = bv_pool.tile([P, lw], F32, tag="bv")
                nc.gpsimd.tensor_tensor(bv_sb[:], s_sb[:], u_sb[:], OP.mult)

                h_sb = h_pool.tile([P, lw], F32, tag=f"h{c}")
                if first:
                    init = h0_sb[:, c:c + 1]
                else:
                    init = h_prev[c][:, h_prev[c].shape[1] - 1:]
                nc.vector.tensor_tensor_scan(
                    h_sb[:], a_sb[:], bv_sb[:], init, OP.mult, OP.add)
                h_prev[c] = h_sb
                nc.sync.dma_start(ht[c * P:(c + 1) * P, l0:l0 + lw], h_sb[:])

            h_prev = [None] * HC
            for ti, (l0, lw) in enumerate(sched):
                x_sbs = x_first if ti == 0 else load_x(l0, lw)

                for c in range(HC):
                    z_ps = psum.tile([P, lw], F32, tag="zps")
                    u_ps = psum.tile([P, lw], F32, tag="ups")
                    for ko in range(KC):
                        nc.tensor.matmul(
                            z_ps[:], w_z[ko][:, c * P:(c + 1) * P],
                            x_sbs[ko][:],
                            start=(ko == 0), stop=(ko == KC - 1))
                    for ko in range(KC):
                        nc.tensor.matmul(
                            u_ps[:], w_u[ko][:, c * P:(c + 1) * P],
                            x_sbs[ko][:],
                            start=(ko == 0), stop=(ko == KC - 1))

                    epilogue(c, l0, lw, z_ps, u_ps, ti == 0)

    split_waits(nc)
    return nc


_program_cache = {}


def _get_program():
    if "nc" not in _program_cache:
        _program_cache["nc"] = build_program()
    return _program_cache["nc"]


def build_in_maps(x, W, b, hx):
    wt = np.ascontiguousarray(W.T.astype(ml_dtypes.bfloat16))
    bz = np.ascontiguousarray(b[:H].reshape(HC, P).T)
    bh = np.ascontiguousarray(b[H:].reshape(HC, P).T)
    in_maps = []
    for n in range(N_CORES):
        in_maps.append({
            "xt": np.ascontiguousarray(x[n].T.astype(ml_dtypes.bfloat16)),
            "wt": wt,
            "bz": bz,
            "bh": bh,
            "h0": np.ascontiguousarray(hx[n].reshape(HC, P).T),
        })
    return in_maps


def kernel(x, W, b, hx, _debug_result=None):
    x = np.ascontiguousarray(x, dtype=np.float32)
    W = np.ascontiguousarray(W, dtype=np.float32)
    b = np.ascontiguousarray(b, dtype=np.float32)
    hx = np.ascontiguousarray(hx, dtype=np.float32)
    N = x.shape[0]
    assert x.shape == (N_CORES, L, HIN) and W.shape == (2 * H, HIN)

    nc = _get_program()
    in_maps = build_in_maps(x, W, b, hx)

    res = run_bass_kernel_spmd(nc, in_maps, core_ids=list(range(N_CORES)))
    if _debug_result is not None:
        _debug_result.append(res)

    out = np.empty((N_CORES, L, H), np.float32)
    for n in range(N_CORES):
        out[n] = res.results[n]["ht"].T
    return out


if __name__ == "__main__":
    rng = np.random.default_rng(0)
    x = rng.standard_normal((N_CORES, L, HIN), dtype=np.float32)
    W = rng.standard_normal((2 * H, HIN), dtype=np.float32) / np.sqrt(HIN)
    b = (rng.standard_normal(2 * H) * 0.01).astype(np.float32)
    hx = rng.random((N_CORES, H), dtype=np.float32)
    out = kernel(x, W, b, hx)
    print("ran ok", out.shape, out.dtype, float(np.abs(out).max()))

